# revision 52
# baseline (speedup 1.0000x reference)
# Trainium2 Bass kernel for nn_BatchelorAdj (motion-compensated MRI recon adjoint).
#
# Math:  out = sum_t W_t^T( sum_c conj(S_c) . IFFT2c(K_c . M_ct) )
#   - IFFT2c(X) == A @ X @ A with A = P F^-1 P (P = fftshift perm, A symmetric),
#     run as float32r matmuls (full fp32 precision, full PE rate at N=320).
#   - W_t^T (adjoint bilinear warp) == sum over 16x8 source tiles of banded
#     matmuls (Ex*im)^T @ Ey where Ex[q,j] = relu(1-|j - pxr_q|) is the exact
#     bilinear weight (triangular hat), built with one subtract + Abs + Relu.
#
# Sharding: 8 cores; core r does frames [3r,3r+1,3r+2] fully, plus coils
# [2r,2r+1] of frame 24 (warp is linear in the image, so per-core partial coil
# sums warp independently and everything adds in the final host-side reduce).
import math
import numpy as np

Nx = Ny = 320
Nc = 16
Nt = 25
NCORES = 8
BX, BY = 32, 4              # warp source tile (BX*BY = 128 = one K chunk)
NTX, NTY = Nx // BX, Ny // BY
NTILE = NTX * NTY           # 800
FR_FULL = 3                 # full frames per core
C24 = Nc // NCORES          # coils of frame 24 per core
NSLOT = FR_FULL + 1

_CACHE = {}


def _build_A():
    j = np.arange(Nx)
    F = np.exp(2j * np.pi * np.outer(j, j) / Nx) / np.sqrt(Nx)
    P = np.zeros((Nx, Nx))
    P[j, (j + Nx // 2) % Nx] = 1.0
    A = P @ F @ P
    return A.real.astype(np.float32), A.imag.astype(np.float32)


def _chunk3(arr2d):
    """[320, W] -> [3, 128, W] zero-padded."""
    out = np.zeros((3, 128, arr2d.shape[1]), dtype=arr2d.dtype)
    out[0] = arr2d[0:128]
    out[1] = arr2d[128:256]
    out[2, :64] = arr2d[256:320]
    return out


def _build_program(D, debug_dump=False):
    from concourse import bass, bacc, tile, mybir

    JX = BX + 2 * D + 1
    JY = BY + 2 * D + 1
    PWW = BY * (NTY - 1) + JY          # psum band width (357 for D=18)
    f32 = mybir.dt.float32
    f32r = mybir.dt.float32r
    bf16 = mybir.dt.bfloat16
    MULT = mybir.AluOpType.mult
    ADD = mybir.AluOpType.add
    SUB = mybir.AluOpType.subtract
    ACTF = mybir.ActivationFunctionType

    nc = bacc.Bacc("TRN2", target_bir_lowering=False, debug=False,
                   num_devices=NCORES)

    # ---- DRAM tensors (SPMD: same shapes on all cores, per-core values) ----
    ksT = nc.dram_tensor("ksT", [Nc, 2, 3, 128, Nx], f32, kind="ExternalInput")
    ks24T = nc.dram_tensor("ks24T", [C24, 2, 3, 128, Nx], f32, kind="ExternalInput")
    maskT = nc.dram_tensor("maskT", [FR_FULL, Nc, 3, 128, Nx], bf16, kind="ExternalInput")
    mask24T = nc.dram_tensor("mask24T", [C24, 3, 128, Nx], bf16, kind="ExternalInput")
    smg = nc.dram_tensor("smg", [Nc, 2, 3, 128, Ny], f32, kind="ExternalInput")
    sm24g = nc.dram_tensor("sm24g", [C24, 2, 3, 128, Ny], f32, kind="ExternalInput")
    Acst = nc.dram_tensor("Acst", [3, 3, 128, Ny], f32r, kind="ExternalInput")
    pxrd = nc.dram_tensor("pxrd", [NSLOT, 128, NTILE], f32, kind="ExternalInput")
    pyrd = nc.dram_tensor("pyrd", [NSLOT, 128, NTILE], f32, kind="ExternalInput")
    iotaxd = nc.dram_tensor("iotaxd", [128, JX], f32, kind="ExternalInput")
    iotayd = nc.dram_tensor("iotayd", [128, JY], f32, kind="ExternalInput")
    zzd = nc.dram_tensor("zzd", [1, 512], f32, kind="ExternalInput")
    outp = nc.dram_tensor("outp", [2, 3, 128, Ny], f32, kind="ExternalOutput")
    if debug_dump:
        dbg_aux = nc.dram_tensor("dbg_aux", [2, 3, 128, Ny], f32, kind="ExternalOutput")
        dbg_imc = nc.dram_tensor("dbg_imc", [2, 128, NTILE], f32, kind="ExternalOutput")
        dbg_t1 = nc.dram_tensor("dbg_t1", [2, 3, 128, Ny], f32, kind="ExternalOutput")

    from contextlib import ExitStack
    with tile.TileContext(nc) as tc, ExitStack() as ctx:
        const_pool = ctx.enter_context(tc.tile_pool(name="const", bufs=1))
        acc_pool = ctx.enter_context(tc.tile_pool(name="acc", bufs=1))
        aux_pool = ctx.enter_context(tc.tile_pool(name="aux", bufs=2))
        ks_pool = ctx.enter_context(tc.tile_pool(name="ks", bufs=2))
        mk_pool = ctx.enter_context(tc.tile_pool(name="mk", bufs=3))
        km_pool = ctx.enter_context(tc.tile_pool(name="km", bufs=2))
        t1_pool = ctx.enter_context(tc.tile_pool(name="t1", bufs=2))
        sm_pool = ctx.enter_context(tc.tile_pool(name="sm", bufs=2))
        cmb_pool = ctx.enter_context(tc.tile_pool(name="cmb", bufs=4))
        imc_pool = ctx.enter_context(tc.tile_pool(name="imc", bufs=2))
        pxy_pool = ctx.enter_context(tc.tile_pool(name="pxy", bufs=1))
        ex_pool = ctx.enter_context(tc.tile_pool(name="ex", bufs=2))
        ey_pool = ctx.enter_context(tc.tile_pool(name="ey", bufs=2))
        exim_pool = ctx.enter_context(tc.tile_pool(name="exim", bufs=2))
        fl_pool = ctx.enter_context(tc.tile_pool(name="fl", bufs=2))
        psum_fft = ctx.enter_context(tc.tile_pool(name="psf", bufs=6, space="PSUM"))
        psum_warp = ctx.enter_context(tc.tile_pool(name="psw", bufs=1, space="PSUM"))

        # ---- constants to SBUF ----
        A_sb = []
        for w in range(3):
            t = const_pool.tile([128, 3, Ny], f32r, tag=f"A{w}")
            nc.sync.dma_start(t[:, :, :], Acst.ap()[w].transpose([1, 0, 2]))
            A_sb.append(t)
        iotax_sb = const_pool.tile([128, JX], f32, tag="iox")
        nc.sync.dma_start(iotax_sb[:, :], iotaxd.ap()[:, :])
        iotay_sb = const_pool.tile([128, JY], f32, tag="ioy")
        nc.sync.dma_start(iotay_sb[:, :], iotayd.ap()[:, :])
        zz_sb = const_pool.tile([1, 512], f32, tag="zz")
        nc.sync.dma_start(zz_sb[:, :], zzd.ap()[:, :])
        zzh_sb = const_pool.tile([1, 512], bf16, tag="zzh")
        nc.vector.tensor_copy(zzh_sb[:, :], zz_sb[:, :])

        out_acc = []
        for comp in range(2):
            t = acc_pool.tile([128, 3, Ny], f32, tag=f"oacc{comp}")
            nc.vector.memset(t[:, :, :], 0.0)
            out_acc.append(t)

        AR, AI, NAI = 0, 1, 2

        # ---- FFT + coil-combine; coils OUTER so kspace/smaps stream twice
        # per core instead of once per frame; slots processed in pairs to
        # bound live aux accumulators ----

        def fft_combine(ksbt, smbt, mk_src, auxp):
            msb = mk_pool.tile([128, 3, Nx], bf16, tag="msb")
            nc.sync.dma_start(msb[:, :, :], mk_src.transpose([1, 0, 2]))
            kmTt = []
            for comp in range(2):
                km = km_pool.tile([128, 3, Nx], f32r, tag=f"km{comp}",
                                  name=f"km_{comp}")
                nc.gpsimd.tensor_tensor(km[:, :, :], ksbt[comp][:, :, :],
                                        msb[:, :, :], MULT)
                kmTt.append(km)

            # pass 1: T1 = km @ A  (T1[x,u], stored [p, m, u])
            T1 = [t1_pool.tile([128, 3, Ny], f32r, tag=f"T1{comp}",
                               name=f"T1_{comp}")
                  for comp in range(2)]
            for m in range(3):
                mc = 128 if m < 2 else 64
                for comp, terms in ((0, ((0, AR), (1, NAI))),
                                    (1, ((0, AI), (1, AR)))):
                    ps = psum_fft.tile([128, Ny], f32, tag="psf", name="psf")
                    i = 0
                    for (kcomp, w) in terms:
                        for ky in range(3):
                            kp = 128 if ky < 2 else 64
                            nc.tensor.matmul(
                                ps[0:mc, :],
                                kmTt[kcomp][0:kp, ky, 128 * m:128 * m + mc],
                                A_sb[w][0:kp, ky, :],
                                start=(i == 0), stop=(i == 5))
                            i += 1
                    nc.vector.tensor_copy(T1[comp][0:mc, m, :], ps[0:mc, :])

            # pass 2: im = A @ T1 ; combine with conj(smaps)
            for m in range(3):
                mc = 128 if m < 2 else 64
                psv = []
                for comp, terms in ((0, ((0, AR), (1, NAI))),
                                    (1, ((1, AR), (0, AI)))):
                    ps = psum_fft.tile([128, Ny], f32, tag="psf", name="psf2")
                    i = 0
                    for (tcomp, w) in terms:
                        for kx in range(3):
                            kp = 128 if kx < 2 else 64
                            nc.tensor.matmul(
                                ps[0:mc, :],
                                A_sb[w][0:kp, kx, 128 * m:128 * m + mc],
                                T1[tcomp][0:kp, kx, :],
                                start=(i == 0), stop=(i == 5))
                            i += 1
                    psv.append(ps)
                # aux_r += sr*imr + si*imi ; aux_i += sr*imi - si*imr
                for (ocomp, scomp, icomp, op) in ((0, 0, 0, ADD), (0, 1, 1, ADD),
                                                 (1, 0, 1, ADD), (1, 1, 0, SUB)):
                    p = cmb_pool.tile([128, Ny], f32, tag="cmb", name="cmb")
                    nc.vector.tensor_tensor(p[0:mc, :], smbt[scomp][0:mc, m, :],
                                            psv[icomp][0:mc, :], MULT)
                    nc.vector.tensor_tensor(auxp[ocomp][0:mc, m, :],
                                            auxp[ocomp][0:mc, m, :], p[0:mc, :], op)

        def load_ks_sm(ks_src, sm_src):
            ksbt, smbt = [], []
            for comp in range(2):
                ksb = ks_pool.tile([128, 3, Nx], f32, tag=f"ksb{comp}",
                                   name=f"ksb_{comp}")
                nc.sync.dma_start(ksb[:, :, :], ks_src[comp].transpose([1, 0, 2]))
                ksbt.append(ksb)
                smb = sm_pool.tile([128, 3, Ny], f32, tag=f"smb{comp}",
                                   name=f"smb_{comp}")
                nc.sync.dma_start(smb[:, :, :], sm_src[comp].transpose([1, 0, 2]))
                smbt.append(smb)
            return ksbt, smbt

        for grp in ([0, 1], [2, 3]):
          aux = {}
          for slot in grp:
            pair = []
            for comp in range(2):
                t = aux_pool.tile([128, 3, Ny], f32, tag=f"aux{slot % 2}{comp}",
                                  name=f"aux_{slot}_{comp}")
                nc.vector.memset(t[:, :, :], 0.0)
                pair.append(t)
            aux[slot] = pair
          for c in range(Nc):
            ksbt, smbt = load_ks_sm(ksT.ap()[c], smg.ap()[c])
            for slot in grp:
                if slot < FR_FULL:
                    fft_combine(ksbt, smbt, maskT.ap()[slot, c], aux[slot])
          if 3 in grp:
            for c in range(C24):
                ksbt, smbt = load_ks_sm(ks24T.ap()[c], sm24g.ap()[c])
                fft_combine(ksbt, smbt, mask24T.ap()[c], aux[3])

          # ---- phase 2: adjoint warp per slot ----
          for slot in grp:
            if debug_dump and slot == 0:
                for comp in range(2):
                    nc.sync.dma_start(dbg_aux.ap()[comp].transpose([1, 0, 2]),
                                      aux[slot][comp][:, :, :])
            imc = []
            for comp in range(2):
                t = imc_pool.tile([128, NTILE], f32, tag=f"imc{comp}",
                                  name=f"imc_{comp}")
                with nc.allow_non_contiguous_dma(reason="strided imc gather"):
                    for a in range(NTX):
                        k, p0 = (32 * a) // 128, (32 * a) % 128
                        rs = aux[slot][comp][p0:p0 + 32, k, :].rearrange(
                            "p (g ul) -> p g ul", g=NTY, ul=BY)
                        for ul in range(BY):
                            nc.sync.dma_start(
                                t[32 * ul:32 * ul + 32, NTY * a:NTY * a + NTY],
                                rs[:, :, ul])
                imc.append(t)
            if debug_dump and slot == 0:
                for comp in range(2):
                    nc.sync.dma_start(dbg_imc.ap()[comp], imc[comp][:, :])

            pxr_sb = pxy_pool.tile([128, NTILE], f32, tag="pxr")
            nc.sync.dma_start(pxr_sb[:, :], pxrd.ap()[slot])
            pyr_sb = pxy_pool.tile([128, NTILE], f32, tag="pyr")
            nc.sync.dma_start(pyr_sb[:, :], pyrd.ap()[slot])

            for bx in range(NTX):
                pw = []
                for comp in range(2):
                    t = psum_warp.tile([JX, PWW], f32, tag=f"pw{comp}",
                                       name=f"pw_{comp}")
                    nc.tensor.matmul(t[:, :], zzh_sb[0:1, 0:JX], zzh_sb[0:1, 0:PWW],
                                     start=True, stop=False, skip_group_check=True)
                    pw.append(t)

                NH = NTY // 2     # construction sub-chunk (SBUF pressure)
                for h in range(2):
                    c0 = NTY * bx + NH * h
                    eng = nc.gpsimd if h == 0 else nc.vector
                    ex = ex_pool.tile([128, NH, JX], f32, tag="ex")
                    eng.tensor_tensor(
                        ex[:, :, :],
                        iotax_sb[:, :].unsqueeze(1).broadcast_to([128, NH, JX]),
                        pxr_sb[:, c0:c0 + NH].unsqueeze(2).broadcast_to([128, NH, JX]),
                        SUB)
                    ey = ey_pool.tile([128, NH, JY], f32, tag="ey")
                    eng.tensor_tensor(
                        ey[:, :, :],
                        iotay_sb[:, :].unsqueeze(1).broadcast_to([128, NH, JY]),
                        pyr_sb[:, c0:c0 + NH].unsqueeze(2).broadcast_to([128, NH, JY]),
                        SUB)
                    nc.scalar.activation(ex[:, :, :], ex[:, :, :], ACTF.Abs)
                    nc.scalar.activation(ey[:, :, :], ey[:, :, :], ACTF.Abs)
                    nc.scalar.activation(ex[:, :, :], ex[:, :, :], ACTF.Relu,
                                         scale=-1.0, bias=1.0)
                    nc.scalar.activation(ey[:, :, :], ey[:, :, :], ACTF.Relu,
                                         scale=-1.0, bias=1.0)
                    eyim = []
                    for comp in range(2):
                        t = exim_pool.tile([128, NH, JY], f32, tag=f"eyim{comp}",
                                           name=f"eyim_{comp}")
                        e2 = nc.vector if comp == 0 else nc.gpsimd
                        e2.tensor_tensor(
                            t[:, :, :], ey[:, :, :],
                            imc[comp][:, c0:c0 + NH].unsqueeze(2)
                            .broadcast_to([128, NH, JY]),
                            MULT)
                        eyim.append(t)

                    for ti in range(NH):
                        y0 = BY * (NH * h + ti)
                        for comp in range(2):
                            nc.tensor.matmul(
                                pw[comp][:, y0:y0 + JY],
                                ex[:, ti, :],
                                eyim[comp][:, ti, :],
                                start=False,
                                stop=(h == 1 and ti == NH - 1),
                                skip_group_check=True)

                # ---- flush band: rows [BX*bx - D, BX*bx + BX + D] ----
                g0 = BX * bx - D
                r0, r1 = max(0, g0), min(Nx, g0 + JX)
                for comp in range(2):
                    tmp = fl_pool.tile([JX, Ny], f32, tag="fl")
                    nc.vector.tensor_copy(tmp[0:JX, :], pw[comp][0:JX, D:D + Ny])
                    ra = r0
                    while ra < r1:
                        k = ra // 128
                        rb = min(r1, 128 * (k + 1))
                        pa, pb = ra - 128 * k, rb - 128 * k
                        tmp2 = fl_pool.tile([128, Ny], f32, tag="fl2")
                        nc.gpsimd.memset(tmp2[:, :], 0.0)
                        nc.sync.dma_start(tmp2[pa:pb, :], tmp[ra - g0:rb - g0, :])
                        nc.vector.tensor_tensor(
                            out_acc[comp][:, k, :],
                            out_acc[comp][:, k, :], tmp2[:, :], ADD)
                        ra = rb

        for comp in range(2):
            nc.sync.dma_start(outp.ap()[comp].transpose([1, 0, 2]),
                              out_acc[comp][:, :, :])

    nc.compile()
    return nc


def _host_prep(kspace_r, kspace_i, mask, smaps_r, smaps_i, flow, D):
    f32 = np.float32
    import ml_dtypes
    bf16 = ml_dtypes.bfloat16
    JX = BX + 2 * D + 1
    JY = BY + 2 * D + 1

    Ar, Ai = _build_A()
    Acst = np.stack([_chunk3(Ar), _chunk3(Ai), _chunk3(-Ai)])  # [3,3,128,320]

    # kspace transposed [c, comp, ychunk, p, x]
    kT = np.stack([kspace_r.transpose(2, 1, 0), kspace_i.transpose(2, 1, 0)], 1)
    ksT = np.zeros((Nc, 2, 3, 128, Nx), f32)
    ksT[:, :, 0] = kT[:, :, 0:128]
    ksT[:, :, 1] = kT[:, :, 128:256]
    ksT[:, :, 2, :64] = kT[:, :, 256:320]

    # mask transposed [t, c, ychunk, p, x] bf16
    mT = mask.transpose(3, 2, 1, 0)  # [t, c, y, x]
    maskT = np.zeros((Nt, Nc, 3, 128, Nx), bf16)
    maskT[:, :, 0] = mT[:, :, 0:128].astype(bf16)
    maskT[:, :, 1] = mT[:, :, 128:256].astype(bf16)
    maskT[:, :, 2, :64] = mT[:, :, 256:320].astype(bf16)

    # smaps natural [c, comp, vchunk, p, u]
    sT = np.stack([smaps_r.transpose(2, 0, 1), smaps_i.transpose(2, 0, 1)], 1)
    smg = np.zeros((Nc, 2, 3, 128, Ny), f32)
    smg[:, :, 0] = sT[:, :, 0:128]
    smg[:, :, 1] = sT[:, :, 128:256]
    smg[:, :, 2, :64] = sT[:, :, 256:320]

    # warp fields: pxr[t, q, tile] = px - BX*bx + D in compact tile layout
    X, Y = np.meshgrid(np.arange(Nx, dtype=f32), np.arange(Ny, dtype=f32),
                       indexing="ij")
    pxr_all = np.zeros((Nt, 128, NTILE), f32)
    pyr_all = np.zeros((Nt, 128, NTILE), f32)
    bxg = np.repeat(np.arange(NTX), NTY).reshape(1, NTILE)  # tile -> bx
    byg = np.tile(np.arange(NTY), NTX).reshape(1, NTILE)
    for t in range(Nt):
        px = np.clip(X + flow[:, :, 0, t], 0.0, Nx - 1.0)
        py = np.clip(Y + flow[:, :, 1, t], 0.0, Ny - 1.0)
        # [bx, xin, by, yin] -> [q = xin*BY + yin, tile = bx*NTY + by]
        # q = vl + 32*ul  (vl = v%BX, ul = u%BY) -> dims order (ul, vl)
        pxc = px.reshape(NTX, BX, NTY, BY).transpose(3, 1, 0, 2).reshape(128, NTILE)
        pyc = py.reshape(NTX, BX, NTY, BY).transpose(3, 1, 0, 2).reshape(128, NTILE)
        pxr_all[t] = pxc - BX * bxg + D
        pyr_all[t] = pyc - BY * byg + D
    assert pxr_all.min() >= 0 and pxr_all.max() <= JX - 1 + 1e-3
    assert pyr_all.min() >= 0 and pyr_all.max() <= JY - 1 + 1e-3

    iotax = np.tile(np.arange(JX, dtype=f32), (128, 1))
    iotay = np.tile(np.arange(JY, dtype=f32), (128, 1))
    zz = np.zeros((1, 512), f32)

    in_maps = []
    for r in range(NCORES):
        fr = [FR_FULL * r + s for s in range(FR_FULL)]
        cs = [C24 * r + j for j in range(C24)]
        in_maps.append({
            "ksT": ksT,
            "ks24T": np.ascontiguousarray(ksT[cs]),
            "maskT": np.ascontiguousarray(maskT[fr]),
            "mask24T": np.ascontiguousarray(maskT[Nt - 1, cs]),
            "smg": smg,
            "sm24g": np.ascontiguousarray(smg[cs]),
            "Acst": Acst,
            "pxrd": np.ascontiguousarray(pxr_all[fr + [Nt - 1]]),
            "pyrd": np.ascontiguousarray(pyr_all[fr + [Nt - 1]]),
            "iotaxd": iotax,
            "iotayd": iotay,
            "zzd": zz,
        })
    return in_maps


def kernel(kspace_r, kspace_i, mask, smaps_r, smaps_i, flow):
    from concourse.bass_utils import run_bass_kernel_spmd

    D = max(17, int(math.ceil(np.abs(flow).max())))
    if D not in _CACHE:
        _CACHE[D] = _build_program(D)
    nc = _CACHE[D]

    in_maps = _host_prep(np.asarray(kspace_r, np.float32),
                         np.asarray(kspace_i, np.float32),
                         np.asarray(mask, np.float32),
                         np.asarray(smaps_r, np.float32),
                         np.asarray(smaps_i, np.float32),
                         np.asarray(flow, np.float32), D)

    res = run_bass_kernel_spmd(nc, in_maps, core_ids=list(range(NCORES)))

    acc = np.zeros((2, Nx, Ny), np.float64)
    for r in range(NCORES):
        o = res.results[r]["outp"].astype(np.float64)  # [2, 3, 128, 320]
        for comp in range(2):
            acc[comp, 0:128] += o[comp, 0]
            acc[comp, 128:256] += o[comp, 1]
            acc[comp, 256:320] += o[comp, 2, :64]
    return np.stack([acc[0], acc[1]], axis=-1).astype(np.float32)
